# revision 27
# baseline (speedup 1.0000x reference)
"""Distributed Trainium2 Bass kernel for nn_Attention_66915590471696.

Sharding: 8 cores, each core owns 2 heads (core c -> heads 2c, 2c+1) and
processes BOTH batches (so attn_bias is loaded once per head across batches).
The out-projection is computed per-core against the owned head rows of Wout;
the host sums the 8 partial outputs (TP-reduce on the host during unshard).

v2 layout/dataflow (vs baseline):
  - attn_bias is pre-TRANSPOSED on the host (layout only) to [h, j, i] tiles,
    so the 1024 PE transpose matmuls are gone; bias tiles DMA to SBUF and are
    added to the QK^T PSUM scores with one DVE tensor_tensor per tile.
  - i is processed in 4 chunks of 512 (outer loop); normalize + out-projection
    + output DMA stream per chunk, overlapping the attention of later chunks.
  - softmax normalizer via ones-column in v_aug (row DH of oacc), reciprocal
    on DVE + PE broadcast, fused normalize-copy into the outproj lhsT.
"""
import sys, os, types, math
sys.path.insert(0, '/opt/trn_rl_repo')
import numpy as np
from contextlib import ExitStack


def _install_axon_hooks_shim():
    try:
        import antenv.axon_hooks  # noqa
        return
    except ImportError:
        pass
    try:
        from trn_agent_boot.trn_boot import _ntff_profile_via_ctypes
        hook = _ntff_profile_via_ctypes('/opt/axon/libaxon_pjrt.so')
    except Exception:
        hook = None
    mod = types.ModuleType('antenv.axon_hooks')
    mod._hook = hook
    mod.get_axon_ntff_profile_hook = lambda: mod._hook
    def set_axon_ntff_profile_hook(h):
        mod._hook = h
    mod.set_axon_ntff_profile_hook = set_axon_ntff_profile_hook
    sys.modules['antenv.axon_hooks'] = mod


_install_axon_hooks_shim()

import concourse.bass as bass
import concourse.tile as tile
from concourse import mybir, bacc
from concourse.masks import make_identity

F32 = mybir.dt.float32
F32R = mybir.dt.float32r
BF16 = mybir.dt.bfloat16

B, N, D, H, DH = 2, 2048, 1024, 16, 64
P = 128
NH = 2               # heads per core
NC = 8               # cores
SCALE = DH ** -0.5
NCH = N // P         # 16 n-chunks
JT = N // P          # 16 j tiles
IQ = 4               # i chunks
IQW = N // IQ        # 512


def build_nc():
    nc = bacc.Bacc("TRN2", target_bir_lowering=False, debug=False)

    xt = nc.declare_dram_parameter("xt", [B, P, D // P, N], F32, isOutput=False)
    wq = nc.declare_dram_parameter("wq", [P, D // P, P], F32, isOutput=False)
    wk = nc.declare_dram_parameter("wk", [P, D // P, P], F32, isOutput=False)
    wv = nc.declare_dram_parameter("wv", [P, D // P, P], F32, isOutput=False)
    wmix = nc.declare_dram_parameter("wmix", [P, D // P, NH], F32, isOutput=False)
    wout = nc.declare_dram_parameter("wout", [P, D], F32, isOutput=False)
    boutp = nc.declare_dram_parameter("boutp", [1, D], F32, isOutput=False)
    rott = nc.declare_dram_parameter("rott", [DH, N], F32, isOutput=False)
    biasTp = nc.declare_dram_parameter("biasTp", [NH, JT, IQ, P, IQW], F32, isOutput=False)
    vrp = nc.declare_dram_parameter("vrp", [B, NH, P, NCH, DH], F32, isOutput=False)
    out = nc.declare_dram_parameter("out", [B, NCH, P, D], F32, isOutput=True)

    with tile.TileContext(nc) as tc:
        with ExitStack() as ctx:
            consts = ctx.enter_context(tc.tile_pool(name="consts", bufs=1))
            wpool = ctx.enter_context(tc.tile_pool(name="wpool", bufs=1))
            proj = ctx.enter_context(tc.tile_pool(name="proj", bufs=1))
            ps = ctx.enter_context(tc.tile_pool(name="ps", bufs=4, space="PSUM"))
            oaccp = ctx.enter_context(tc.tile_pool(name="oaccp", bufs=4, space="PSUM"))

            # ---- weights + x DMAs first (front of the DMA queues) ----
            wq_t = wpool.tile([P, D // P, P], BF16)
            nc.gpsimd.dma_start(wq_t[:], wq[:])
            wk_t = wpool.tile([P, D // P, P], BF16)
            nc.gpsimd.dma_start(wk_t[:], wk[:])
            wv_t = wpool.tile([P, D // P, P], BF16)
            nc.gpsimd.dma_start(wv_t[:], wv[:])
            wmix_t = wpool.tile([P, D // P, NH], BF16)
            nc.gpsimd.dma_start(wmix_t[:], wmix[:])
            wout_t = wpool.tile([P, D], BF16)
            nc.gpsimd.dma_start(wout_t[:], wout[:])

            # attention stream pools allocated BEFORE the transient x pools so
            # bias tiles never overlap the x region: bias DMA can run from t=0
            # without waiting for the gpsimd drain at pctx.close().
            biasb16 = ctx.enter_context(tc.tile_pool(name="biasb16", bufs=4))
            biasbf = ctx.enter_context(tc.tile_pool(name="biasbf", bufs=12))
            tmpp = ctx.enter_context(tc.tile_pool(name="tmpp", bufs=6, side="right"))
            ptp = ctx.enter_context(tc.tile_pool(name="ptp", bufs=8))

            # bias tiles stream in consumption order (iq, jt, hh).  3/4 ride
            # the fast sync HWDGE queue as f32 (consumed by DVE adds); 1/4 are
            # cast to bf16 on the gpsimd software DGE (consumed by PE identity
            # injection) to keep the slow queue lightly loaded.
            PF = 16
            bias_tiles = []

            def bias_on_pe(n):
                return n % 4 == 0

            def issue_bias(n):
                if n >= IQ * JT * NH:
                    return
                iq_, r = divmod(n, JT * NH)
                jt_, hh_ = divmod(r, NH)
                if bias_on_pe(n):
                    t = biasb16.tile([P, IQW], BF16, tag="bias16", name=f"bias{n}")
                    nc.gpsimd.dma_start(t[:], biasTp[hh_, jt_, iq_])
                else:
                    t = biasbf.tile([P, IQW], F32, tag="biasf", name=f"bias{n}")
                    nc.sync.dma_start(t[:], biasTp[hh_, jt_, iq_])
                bias_tiles.append(t)

            pctx = ExitStack()
            xpool = pctx.enter_context(tc.tile_pool(name="xpool", bufs=1))
            ptmp = pctx.enter_context(tc.tile_pool(name="ptmp", bufs=1))

            # x first: it gates the projections, which gate everything
            x_t = [None, None]
            for b in range(B):
                x_t[b] = xpool.tile([P, D // P, N], BF16, tag=f"xt{b}", name=f"x_t{b}")
                nc.gpsimd.dma_start(x_t[b][:], xt[b])

            # value_residual early (unblocks v_aug right after projections)
            vr_t = {}
            for b in range(B):
                for hh in range(NH):
                    vr_t[(b, hh)] = ptmp.tile([P, NCH, DH], BF16, tag=f"vr{b}{hh}",
                                              name=f"vr{b}{hh}")
                    nc.gpsimd.dma_start(vr_t[(b, hh)][:], vrp[b, hh])

            # prefetch the bias stream behind x/vr
            for n in range(PF):
                issue_bias(n)

            # ---- constants ----
            ident_f = consts.tile([P, P], F32)
            make_identity(nc, ident_f[:])
            ident_b = consts.tile([P, P], BF16)
            make_identity(nc, ident_b[:])
            ones_t = consts.tile([P, P], F32)
            nc.vector.memset(ones_t[:], 1.0)

            # rotary -> cosT/sinT [128, N] bf16 (head-duplicated on partitions)
            sinT = consts.tile([P, N], BF16)
            cosT = consts.tile([P, N], BF16)
            for rih in range(2):
                rsl = slice(rih * (N // 2), (rih + 1) * (N // 2))
                rt = ptmp.tile([DH, N // 2], F32, tag="rt")
                nc.sync.dma_start(rt[:], rott[:, rsl])
                wrap = ptmp.tile([DH, N // 2], F32, tag="wrap")
                nc.vector.add_range_wrap(wrap[:], rt[:], 0.0, math.pi, 2 * math.pi)
                nc.scalar.activation(sinT[0:DH, rsl], wrap[:], mybir.ActivationFunctionType.Sin)
                wrap2 = ptmp.tile([DH, N // 2], F32, tag="wrap")
                nc.vector.add_range_wrap(wrap2[:], rt[:], math.pi / 2, math.pi, 2 * math.pi)
                nc.scalar.activation(cosT[0:DH, rsl], wrap2[:], mybir.ActivationFunctionType.Sin)
            nc.vector.tensor_copy(sinT[DH:P, :], sinT[0:DH, :])
            nc.vector.tensor_copy(cosT[DH:P, :], cosT[0:DH, :])
            # sinT_rot: sin with the low half of each head's 64-block negated
            sinT_rot = consts.tile([P, N], BF16)
            nc.vector.tensor_copy(sinT_rot[:], sinT[:])
            for lo in (0, DH):
                nc.vector.tensor_scalar(sinT_rot[lo:lo + 32, :], sinT_rot[lo:lo + 32, :],
                                        -1.0, None, mybir.AluOpType.mult)

            # bout broadcast [128, D] f32
            bout_sb = ptmp.tile([1, D], F32)
            nc.sync.dma_start(bout_sb[:], boutp[:])
            bout_bc = consts.tile([P, D], F32)
            for nf in range(0, D, IQW):
                bb_ps = ps.tile([P, IQW], F32, tag="S")
                nc.tensor.matmul(bb_ps[:], ones_t[:1, :P], bout_sb[:, nf:nf + IQW],
                                 start=True, stop=True)
                nc.vector.tensor_copy(bout_bc[:, nf:nf + IQW], bb_ps[:])

            # ---- projections (both batches) ----
            qt = [None, None]; kt = [None, None]
            mixn = [None, None]
            vt = [None, None]
            for b in range(B):
                qt_raw = ptmp.tile([P, N], BF16, tag="qt_raw")
                kt_raw = ptmp.tile([P, N], BF16, tag="kt_raw")
                vt[b] = ptmp.tile([P, N], BF16, tag=f"vt{b}", name=f"vt{b}")
                mixT = ptmp.tile([NH, N], BF16, tag="mixT", name=f"mixT{b}")
                specs = [("q", wq_t, P, qt_raw), ("k", wk_t, P, kt_raw),
                         ("v", wv_t, P, vt[b]), ("m", wmix_t, NH, mixT)]
                for name, w_t, M, dst in specs:
                    for c in range(IQ):
                        sl = slice(c * IQW, (c + 1) * IQW)
                        pps = ps.tile([P, IQW], F32, tag="S")
                        for kk in range(D // P):
                            nc.tensor.matmul(
                                pps[:M, :], w_t[:, kk, :M], x_t[b][:, kk, sl],
                                start=(kk == 0), stop=(kk == D // P - 1))
                        if name == "q":
                            nc.scalar.mul(dst[:, sl], pps[:, :], SCALE)
                        elif name == "m":
                            nc.scalar.activation(dst[:NH, sl], pps[:NH, :],
                                                 mybir.ActivationFunctionType.Sigmoid)
                        else:
                            nc.scalar.copy(dst[:, sl], pps[:, :])

                # RoPE on qT and kT
                qt[b] = proj.tile([P, N], BF16, tag=f"qt{b}", name=f"qt{b}")
                kt[b] = proj.tile([P, N], BF16, tag=f"kt{b}", name=f"kt{b}")
                for src, dst in ((qt_raw, qt[b]), (kt_raw, kt[b])):
                    rot_t = ptmp.tile([P, N], BF16, tag="rot")
                    for hh in range(NH):
                        lo = hh * DH
                        nc.vector.tensor_copy(rot_t[lo:lo + 32, :], src[lo + 32:lo + 64, :])
                        nc.vector.tensor_copy(rot_t[lo + 32:lo + 64, :], src[lo:lo + 32, :])
                    nc.vector.tensor_tensor(dst[:], src[:], cosT[:], mybir.AluOpType.mult)
                    nc.vector.tensor_tensor(rot_t[:], rot_t[:], sinT_rot[:], mybir.AluOpType.mult)
                    nc.vector.tensor_tensor(dst[:], dst[:], rot_t[:], mybir.AluOpType.add)

                # mix natural [128, NCH, NH] f32 via PE transposes
                mixn[b] = proj.tile([P, NCH, NH], F32, tag=f"mixn{b}", name=f"mixn{b}")
                for t in range(NCH):
                    mps = ps.tile([P, IQW], BF16, tag="S")
                    nc.tensor.matmul(mps[:, :NH], mixT[:NH, t * P:(t + 1) * P], ident_b[:NH, :NH],
                                     is_transpose=True, start=True, stop=True)
                    nc.vector.tensor_copy(mixn[b][:, t, :], mps[:, :NH])

            # ---- v_aug (lerped v + ones column), natural [j, d] per (head, batch) ----
            vaug = {}
            for b in range(B):
                for hh in range(NH):
                    va = proj.tile([P, NCH, DH + 1], BF16, tag=f"va{b}{hh}", name=f"va{b}{hh}")
                    nc.vector.memset(va[:, :, DH:DH + 1], 1.0)
                    vr = vr_t[(b, hh)]
                    for t in range(NCH):
                        vps = ps.tile([P, IQW], BF16, tag="S")
                        lo = hh * DH
                        nc.tensor.matmul(vps[:, :DH], vt[b][lo:lo + DH, t * P:(t + 1) * P],
                                         ident_b[lo:lo + DH, lo:lo + DH], is_transpose=True,
                                         start=True, stop=True)
                        df = ptmp.tile([P, DH], BF16, tag="df")
                        nc.vector.tensor_tensor(df[:], vr[:, t, :], vps[:, :DH], mybir.AluOpType.subtract)
                        nc.vector.scalar_tensor_tensor(va[:, t, :DH], df[:], mixn[b][:, t, hh:hh + 1],
                                                       vps[:, :DH], mybir.AluOpType.mult, mybir.AluOpType.add)
                    vaug[(b, hh)] = va
            pctx.close()

            # ---- tail pools (allocated after x/transients are freed) ----
            otqp = ctx.enter_context(tc.tile_pool(name="otqp", bufs=4))
            finp = ctx.enter_context(tc.tile_pool(name="finp", bufs=3))
            zpool = ctx.enter_context(tc.tile_pool(name="zpool", bufs=2, side="right"))

            # ---- attention, streaming per i-chunk of 512 ----
            # The PE queue is in-order: emit each oacc matmul DEPTH tiles after
            # its scores matmul so the PE never stalls on the DVE-add + ACT-exp
            # round trip for pT.  The normalize/out-proj tail of chunk iq is
            # emitted AFTER the first few groups of chunk iq+1 so its long
            # cross-engine chains never drain the PE pipeline.
            DEPTH = 6
            from collections import deque
            pending = deque()   # (oacc_tile, va, jt, pT)

            def flush_one():
                oa, va, jt_, pT_ = pending.popleft()
                nc.tensor.matmul(oa[:], va[:, jt_, :], pT_[:],
                                 start=(jt_ == 0), stop=(jt_ == JT - 1))

            def emit_tail(iq, oacc):
                for b in range(B):
                    outTq = otqp.tile([P, IQW], BF16, tag="otq")
                    for hh in range(NH):
                        oa = oacc[(b, hh)]
                        zrow = zpool.tile([1, IQW], F32, tag="zrow")
                        nc.vector.tensor_copy(zrow[:], oa[DH:DH + 1, :])
                        rz = zpool.tile([1, IQW], F32, tag="rz")
                        rzs = zpool.tile([1, IQW], F32, tag="rzs")
                        nc.vector.reciprocal_approx_accurate(rz[:], zrow[:], rzs[:])
                        zb = ps.tile([P, IQW], F32, tag="S")
                        nc.tensor.matmul(zb[:DH, :], ones_t[:1, :DH], rz[:],
                                         start=True, stop=True)
                        zbS = zpool.tile([DH, IQW], F32, tag="zbS")
                        nc.scalar.copy(zbS[:], zb[:DH, :])
                        nc.vector.tensor_tensor(outTq[hh * DH:(hh + 1) * DH, :], zbS[:],
                                                oa[0:DH, :], mybir.AluOpType.mult)
                    for it in range(IQW // P):
                        tg = iq * (IQW // P) + it
                        fin = finp.tile([P, D], F32, tag="fin")
                        for df in range(0, D, IQW):
                            pp = ps.tile([P, IQW], F32, tag="S")
                            nc.tensor.matmul(pp[:], outTq[:, it * P:(it + 1) * P],
                                             wout_t[:, df:df + IQW], start=True, stop=True)
                            nc.vector.tensor_tensor(fin[:, df:df + IQW], bout_bc[:, df:df + IQW],
                                                    pp[:], mybir.AluOpType.add)
                        nc.sync.dma_start(out[b, tg], fin[:])

            prev = None  # (iq, oacc) awaiting tail emission
            for iq in range(IQ):
                isl = slice(iq * IQW, (iq + 1) * IQW)
                oacc = {}
                for b in range(B):
                    for hh in range(NH):
                        oacc[(b, hh)] = oaccp.tile([DH + 1, IQW], F32, tag="oacc",
                                                   name=f"oacc{iq}_{b}{hh}")
                for jt in range(JT):
                    for hh in range(NH):
                        bn = (iq * JT + jt) * NH + hh
                        bias_sb = bias_tiles[bn]
                        issue_bias(bn + PF)
                        lo = hh * DH
                        use_pe = bias_on_pe(bn)
                        for b in range(B):
                            # Balance engines: 3/4 of bias tiles add on the
                            # DVE; 1/4 inject into PSUM via a bf16 identity
                            # matmul (1 cycle/row) and exp straight from PSUM.
                            S = ps.tile([P, IQW], F32, tag="S")
                            if use_pe:
                                nc.tensor.matmul(S[:], ident_b[:], bias_sb[:],
                                                 start=True, stop=False,
                                                 skip_group_check=True)
                            nc.tensor.matmul(S[:], kt[b][lo:lo + DH, jt * P:(jt + 1) * P],
                                             qt[b][lo:lo + DH, isl],
                                             start=not use_pe, stop=True,
                                             skip_group_check=True)
                            pT = ptp.tile([P, IQW], BF16, tag="pT")
                            if use_pe:
                                nc.scalar.activation(pT[:], S[:],
                                                     mybir.ActivationFunctionType.Exp)
                            else:
                                tmp = tmpp.tile([P, IQW], F32, tag="tmp")
                                nc.vector.tensor_tensor(tmp[:], bias_sb[:], S[:],
                                                        mybir.AluOpType.add)
                                nc.scalar.activation(pT[:], tmp[:],
                                                     mybir.ActivationFunctionType.Exp)
                            pending.append((oacc[(b, hh)], vaug[(b, hh)], jt, pT))
                            if len(pending) > DEPTH:
                                flush_one()
                    # Deferred tail: must land before the first oacc flush of
                    # this chunk (the oacc PSUM ring aliases the previous
                    # chunk's accumulators).
                    if jt == 0 and prev is not None:
                        emit_tail(*prev)
                        prev = None
                while pending:
                    flush_one()
                prev = (iq, oacc)
            emit_tail(*prev)

    nc.compile()
    return nc


def make_in_maps(x, mask, rotary_emb, attn_bias, value_residual, Wq, Wkv, Wmix, Wout, bout):
    """Shard + lay out the full inputs for the 8 cores. Layout only, no math."""
    x = np.asarray(x); rotary_emb = np.asarray(rotary_emb)
    attn_bias = np.asarray(attn_bias); value_residual = np.asarray(value_residual)
    Wq = np.asarray(Wq); Wkv = np.asarray(Wkv); Wmix = np.asarray(Wmix)
    Wout = np.asarray(Wout); bout = np.asarray(bout)

    xt_pre = np.ascontiguousarray(
        x.transpose(0, 2, 1).reshape(B, D // P, P, N).transpose(0, 2, 1, 3))
    rott = np.ascontiguousarray(rotary_emb.T)

    def wslice(Wcols):  # [1024, 128 or NH] -> [128, 8, M]
        M = Wcols.shape[1]
        return np.ascontiguousarray(Wcols.reshape(D // P, P, M).transpose(1, 0, 2))

    in_maps = []
    for c in range(NC):
        h0 = NH * c
        hs = slice(h0, h0 + NH)
        # bias transposed to [h, j, i], tiled [h, jt, iq, 128(j), 512(i)]
        biasT = attn_bias[hs].transpose(0, 2, 1)  # [NH, j, i]
        biasTp = np.ascontiguousarray(
            biasT.reshape(NH, JT, P, IQ, IQW).transpose(0, 1, 3, 2, 4))
        vrp = np.ascontiguousarray(
            value_residual[:, hs].reshape(B, NH, NCH, P, DH).transpose(0, 1, 3, 2, 4))
        in_maps.append({
            "xt": xt_pre,
            "wq": wslice(Wq[:, h0 * DH:(h0 + NH) * DH]),
            "wk": wslice(Wkv[:, h0 * DH:(h0 + NH) * DH]),
            "wv": wslice(Wkv[:, H * DH + h0 * DH: H * DH + (h0 + NH) * DH]),
            "wmix": wslice(Wmix[:, hs]),
            "wout": np.ascontiguousarray(Wout[h0 * DH:(h0 + NH) * DH, :]),
            "boutp": (bout if c == 0 else np.zeros_like(bout)).reshape(1, D),
            "rott": rott,
            "biasTp": biasTp,
            "vrp": vrp,
        })
    return in_maps


def unshard(results):
    full = np.zeros((B, NCH, P, D), np.float32)
    for r in results:
        full += r["out"]
    return full.reshape(B, N, D)


_NC_CACHE = None


def kernel(**inputs):
    global _NC_CACHE
    from concourse.bass_utils import run_bass_kernel_spmd
    if _NC_CACHE is None:
        _NC_CACHE = build_nc()
    in_maps = make_in_maps(**inputs)
    res = run_bass_kernel_spmd(_NC_CACHE, in_maps, core_ids=list(range(NC)))
    return unshard(res.results)


# revision 28
# speedup vs baseline: 1.0423x; 1.0423x over previous
"""Distributed Trainium2 Bass kernel for nn_Attention_66915590471696.

Sharding: 8 cores, each core owns 2 heads (core c -> heads 2c, 2c+1) and
processes BOTH batches (so attn_bias is loaded once per head across batches).
The out-projection is computed per-core against the owned head rows of Wout;
the host sums the 8 partial outputs (TP-reduce on the host during unshard).

v2 layout/dataflow (vs baseline):
  - attn_bias is pre-TRANSPOSED on the host (layout only) to [h, j, i] tiles,
    so the 1024 PE transpose matmuls are gone; bias tiles DMA to SBUF and are
    added to the QK^T PSUM scores with one DVE tensor_tensor per tile.
  - i is processed in 4 chunks of 512 (outer loop); normalize + out-projection
    + output DMA stream per chunk, overlapping the attention of later chunks.
  - softmax normalizer via ones-column in v_aug (row DH of oacc), reciprocal
    on DVE + PE broadcast, fused normalize-copy into the outproj lhsT.
"""
import sys, os, types, math
sys.path.insert(0, '/opt/trn_rl_repo')
import numpy as np
from contextlib import ExitStack


def _install_axon_hooks_shim():
    try:
        import antenv.axon_hooks  # noqa
        return
    except ImportError:
        pass
    try:
        from trn_agent_boot.trn_boot import _ntff_profile_via_ctypes
        hook = _ntff_profile_via_ctypes('/opt/axon/libaxon_pjrt.so')
    except Exception:
        hook = None
    mod = types.ModuleType('antenv.axon_hooks')
    mod._hook = hook
    mod.get_axon_ntff_profile_hook = lambda: mod._hook
    def set_axon_ntff_profile_hook(h):
        mod._hook = h
    mod.set_axon_ntff_profile_hook = set_axon_ntff_profile_hook
    sys.modules['antenv.axon_hooks'] = mod


_install_axon_hooks_shim()

import concourse.bass as bass
import concourse.tile as tile
from concourse import mybir, bacc
from concourse.masks import make_identity

F32 = mybir.dt.float32
F32R = mybir.dt.float32r
BF16 = mybir.dt.bfloat16

B, N, D, H, DH = 2, 2048, 1024, 16, 64
P = 128
NH = 2               # heads per core
NC = 8               # cores
SCALE = DH ** -0.5
NCH = N // P         # 16 n-chunks
JT = N // P          # 16 j tiles
IQ = 4               # i chunks
IQW = N // IQ        # 512


def build_nc():
    nc = bacc.Bacc("TRN2", target_bir_lowering=False, debug=False)

    xt = nc.declare_dram_parameter("xt", [B, P, D // P, N], F32, isOutput=False)
    wq = nc.declare_dram_parameter("wq", [P, D // P, P], F32, isOutput=False)
    wk = nc.declare_dram_parameter("wk", [P, D // P, P], F32, isOutput=False)
    wv = nc.declare_dram_parameter("wv", [P, D // P, P], F32, isOutput=False)
    wmix = nc.declare_dram_parameter("wmix", [P, D // P, NH], F32, isOutput=False)
    wout = nc.declare_dram_parameter("wout", [P, D], F32, isOutput=False)
    boutp = nc.declare_dram_parameter("boutp", [1, D], F32, isOutput=False)
    rott = nc.declare_dram_parameter("rott", [DH, N], F32, isOutput=False)
    biasTp = nc.declare_dram_parameter("biasTp", [NH, JT, IQ, P, IQW], F32, isOutput=False)
    vrp = nc.declare_dram_parameter("vrp", [B, NH, P, NCH, DH], F32, isOutput=False)
    out = nc.declare_dram_parameter("out", [B, NCH, P, D], F32, isOutput=True)

    with tile.TileContext(nc) as tc:
        with ExitStack() as ctx:
            consts = ctx.enter_context(tc.tile_pool(name="consts", bufs=1))
            wpool = ctx.enter_context(tc.tile_pool(name="wpool", bufs=1))
            proj = ctx.enter_context(tc.tile_pool(name="proj", bufs=1))
            ps = ctx.enter_context(tc.tile_pool(name="ps", bufs=4, space="PSUM"))
            oaccp = ctx.enter_context(tc.tile_pool(name="oaccp", bufs=4, space="PSUM"))

            # ---- weights + x DMAs first (front of the DMA queues) ----
            wq_t = wpool.tile([P, D // P, P], BF16)
            nc.gpsimd.dma_start(wq_t[:], wq[:])
            wk_t = wpool.tile([P, D // P, P], BF16)
            nc.gpsimd.dma_start(wk_t[:], wk[:])
            wv_t = wpool.tile([P, D // P, P], BF16)
            nc.gpsimd.dma_start(wv_t[:], wv[:])
            wmix_t = wpool.tile([P, D // P, NH], BF16)
            nc.gpsimd.dma_start(wmix_t[:], wmix[:])
            wout_t = wpool.tile([P, D], BF16)
            nc.gpsimd.dma_start(wout_t[:], wout[:])

            # attention stream pools allocated BEFORE the transient x pools so
            # bias tiles never overlap the x region: bias DMA can run from t=0
            # without waiting for the gpsimd drain at pctx.close().
            biasb16 = ctx.enter_context(tc.tile_pool(name="biasb16", bufs=4))
            biasbf = ctx.enter_context(tc.tile_pool(name="biasbf", bufs=12))
            tmpp = ctx.enter_context(tc.tile_pool(name="tmpp", bufs=6, side="right"))
            ptp = ctx.enter_context(tc.tile_pool(name="ptp", bufs=8))

            # bias tiles stream in consumption order (iq, jt, hh).  3/4 ride
            # the fast sync HWDGE queue as f32 (consumed by DVE adds); 1/4 are
            # cast to bf16 on the gpsimd software DGE (consumed by PE identity
            # injection) to keep the slow queue lightly loaded.
            PF = 16
            bias_tiles = []

            def bias_on_pe(n):
                return n % 4 == 0

            def issue_bias(n):
                if n >= IQ * JT * NH:
                    return
                iq_, r = divmod(n, JT * NH)
                jt_, hh_ = divmod(r, NH)
                if bias_on_pe(n):
                    t = biasb16.tile([P, IQW], BF16, tag="bias16", name=f"bias{n}")
                    nc.gpsimd.dma_start(t[:], biasTp[hh_, jt_, iq_])
                else:
                    t = biasbf.tile([P, IQW], F32, tag="biasf", name=f"bias{n}")
                    nc.sync.dma_start(t[:], biasTp[hh_, jt_, iq_])
                bias_tiles.append(t)

            pctx = ExitStack()
            xpool = pctx.enter_context(tc.tile_pool(name="xpool", bufs=1))
            ptmp = pctx.enter_context(tc.tile_pool(name="ptmp", bufs=1))

            # x first: it gates the projections, which gate everything
            x_t = [None, None]
            for b in range(B):
                x_t[b] = xpool.tile([P, D // P, N], BF16, tag=f"xt{b}", name=f"x_t{b}")
                nc.gpsimd.dma_start(x_t[b][:], xt[b])

            # value_residual early (unblocks v_aug right after projections)
            vr_t = {}
            for b in range(B):
                for hh in range(NH):
                    vr_t[(b, hh)] = ptmp.tile([P, NCH, DH], BF16, tag=f"vr{b}{hh}",
                                              name=f"vr{b}{hh}")
                    nc.gpsimd.dma_start(vr_t[(b, hh)][:], vrp[b, hh])

            # prefetch the bias stream behind x/vr
            for n in range(PF):
                issue_bias(n)

            # ---- constants ----
            ident_f = consts.tile([P, P], F32)
            make_identity(nc, ident_f[:])
            ident_b = consts.tile([P, P], BF16)
            make_identity(nc, ident_b[:])
            ones_t = consts.tile([P, P], F32)
            nc.vector.memset(ones_t[:], 1.0)

            # rotary -> cosT/sinT [128, N] bf16 (head-duplicated on partitions)
            sinT = consts.tile([P, N], BF16)
            cosT = consts.tile([P, N], BF16)
            for rih in range(2):
                rsl = slice(rih * (N // 2), (rih + 1) * (N // 2))
                rt = ptmp.tile([DH, N // 2], F32, tag="rt")
                nc.sync.dma_start(rt[:], rott[:, rsl])
                wrap = ptmp.tile([DH, N // 2], F32, tag="wrap")
                nc.vector.add_range_wrap(wrap[:], rt[:], 0.0, math.pi, 2 * math.pi)
                nc.scalar.activation(sinT[0:DH, rsl], wrap[:], mybir.ActivationFunctionType.Sin)
                wrap2 = ptmp.tile([DH, N // 2], F32, tag="wrap")
                nc.vector.add_range_wrap(wrap2[:], rt[:], math.pi / 2, math.pi, 2 * math.pi)
                nc.scalar.activation(cosT[0:DH, rsl], wrap2[:], mybir.ActivationFunctionType.Sin)
            nc.vector.tensor_copy(sinT[DH:P, :], sinT[0:DH, :])
            nc.vector.tensor_copy(cosT[DH:P, :], cosT[0:DH, :])
            # sinT_rot: sin with the low half of each head's 64-block negated
            sinT_rot = consts.tile([P, N], BF16)
            nc.vector.tensor_copy(sinT_rot[:], sinT[:])
            for lo in (0, DH):
                nc.vector.tensor_scalar(sinT_rot[lo:lo + 32, :], sinT_rot[lo:lo + 32, :],
                                        -1.0, None, mybir.AluOpType.mult)

            # bout broadcast [128, D] f32
            bout_sb = ptmp.tile([1, D], F32)
            nc.sync.dma_start(bout_sb[:], boutp[:])
            bout_bc = consts.tile([P, D], F32)
            for nf in range(0, D, IQW):
                bb_ps = ps.tile([P, IQW], F32, tag="S")
                nc.tensor.matmul(bb_ps[:], ones_t[:1, :P], bout_sb[:, nf:nf + IQW],
                                 start=True, stop=True)
                nc.vector.tensor_copy(bout_bc[:, nf:nf + IQW], bb_ps[:])

            # ---- projections (both batches) ----
            qt = [None, None]; kt = [None, None]
            mixn = [None, None]
            vt = [None, None]
            for b in range(B):
                qt_raw = ptmp.tile([P, N], BF16, tag="qt_raw")
                kt_raw = ptmp.tile([P, N], BF16, tag="kt_raw")
                vt[b] = ptmp.tile([P, N], BF16, tag=f"vt{b}", name=f"vt{b}")
                mixT = ptmp.tile([NH, N], BF16, tag="mixT", name=f"mixT{b}")
                specs = [("q", wq_t, P, qt_raw), ("k", wk_t, P, kt_raw),
                         ("v", wv_t, P, vt[b]), ("m", wmix_t, NH, mixT)]
                for name, w_t, M, dst in specs:
                    for c in range(IQ):
                        sl = slice(c * IQW, (c + 1) * IQW)
                        pps = ps.tile([P, IQW], F32, tag="S")
                        for kk in range(D // P):
                            nc.tensor.matmul(
                                pps[:M, :], w_t[:, kk, :M], x_t[b][:, kk, sl],
                                start=(kk == 0), stop=(kk == D // P - 1))
                        if name == "q":
                            nc.scalar.mul(dst[:, sl], pps[:, :], SCALE)
                        elif name == "m":
                            nc.scalar.activation(dst[:NH, sl], pps[:NH, :],
                                                 mybir.ActivationFunctionType.Sigmoid)
                        else:
                            nc.scalar.copy(dst[:, sl], pps[:, :])

                # RoPE on qT and kT
                qt[b] = proj.tile([P, N], BF16, tag=f"qt{b}", name=f"qt{b}")
                kt[b] = proj.tile([P, N], BF16, tag=f"kt{b}", name=f"kt{b}")
                for src, dst in ((qt_raw, qt[b]), (kt_raw, kt[b])):
                    rot_t = ptmp.tile([P, N], BF16, tag="rot")
                    for hh in range(NH):
                        lo = hh * DH
                        nc.vector.tensor_copy(rot_t[lo:lo + 32, :], src[lo + 32:lo + 64, :])
                        nc.vector.tensor_copy(rot_t[lo + 32:lo + 64, :], src[lo:lo + 32, :])
                    nc.vector.tensor_tensor(dst[:], src[:], cosT[:], mybir.AluOpType.mult)
                    nc.vector.tensor_tensor(rot_t[:], rot_t[:], sinT_rot[:], mybir.AluOpType.mult)
                    nc.vector.tensor_tensor(dst[:], dst[:], rot_t[:], mybir.AluOpType.add)

                # mix natural [128, NCH, NH] f32 via PE transposes
                mixn[b] = proj.tile([P, NCH, NH], F32, tag=f"mixn{b}", name=f"mixn{b}")
                for t in range(NCH):
                    mps = ps.tile([P, IQW], BF16, tag="S")
                    nc.tensor.matmul(mps[:, :NH], mixT[:NH, t * P:(t + 1) * P], ident_b[:NH, :NH],
                                     is_transpose=True, start=True, stop=True)
                    nc.vector.tensor_copy(mixn[b][:, t, :], mps[:, :NH])

            # ---- v_aug (lerped v + ones column), natural [j, d] per (head, batch) ----
            vaug = {}
            for b in range(B):
                for hh in range(NH):
                    va = proj.tile([P, NCH, DH + 1], BF16, tag=f"va{b}{hh}", name=f"va{b}{hh}")
                    nc.vector.memset(va[:, :, DH:DH + 1], 1.0)
                    vr = vr_t[(b, hh)]
                    for t in range(NCH):
                        vps = ps.tile([P, IQW], BF16, tag="S")
                        lo = hh * DH
                        nc.tensor.matmul(vps[:, :DH], vt[b][lo:lo + DH, t * P:(t + 1) * P],
                                         ident_b[lo:lo + DH, lo:lo + DH], is_transpose=True,
                                         start=True, stop=True)
                        df = ptmp.tile([P, DH], BF16, tag="df")
                        nc.vector.tensor_tensor(df[:], vr[:, t, :], vps[:, :DH], mybir.AluOpType.subtract)
                        nc.vector.scalar_tensor_tensor(va[:, t, :DH], df[:], mixn[b][:, t, hh:hh + 1],
                                                       vps[:, :DH], mybir.AluOpType.mult, mybir.AluOpType.add)
                    vaug[(b, hh)] = va
            pctx.close()

            # ---- tail pools (allocated after x/transients are freed) ----
            otqp = ctx.enter_context(tc.tile_pool(name="otqp", bufs=4))
            finp = ctx.enter_context(tc.tile_pool(name="finp", bufs=3))
            zpool = ctx.enter_context(tc.tile_pool(name="zpool", bufs=2, side="right"))

            # ---- attention, streaming per i-chunk of 512 ----
            # The PE queue is in-order: emit each oacc matmul DEPTH tiles after
            # its scores matmul so the PE never stalls on the DVE-add + ACT-exp
            # round trip for pT.  The normalize/out-proj tail of chunk iq is
            # emitted AFTER the first few groups of chunk iq+1 so its long
            # cross-engine chains never drain the PE pipeline.
            DEPTH = 6
            from collections import deque
            pending = deque()   # (oacc_tile, va, jt, pT)

            def flush_one():
                oa, va, jt_, pT_ = pending.popleft()
                nc.tensor.matmul(oa[:], va[:, jt_, :], pT_[:],
                                 start=(jt_ == 0), stop=(jt_ == JT - 1))

            def emit_tail(iq, oacc):
                for b in range(B):
                    outTq = otqp.tile([P, IQW], BF16, tag="otq")
                    for hh in range(NH):
                        oa = oacc[(b, hh)]
                        zrow = zpool.tile([1, IQW], F32, tag="zrow")
                        nc.vector.tensor_copy(zrow[:], oa[DH:DH + 1, :])
                        rz = zpool.tile([1, IQW], F32, tag="rz")
                        rzs = zpool.tile([1, IQW], F32, tag="rzs")
                        nc.vector.reciprocal_approx_accurate(rz[:], zrow[:], rzs[:])
                        zb = ps.tile([P, IQW], F32, tag="S")
                        nc.tensor.matmul(zb[:DH, :], ones_t[:1, :DH], rz[:],
                                         start=True, stop=True)
                        zbS = zpool.tile([DH, IQW], F32, tag="zbS")
                        nc.scalar.copy(zbS[:], zb[:DH, :])
                        nc.vector.tensor_tensor(outTq[hh * DH:(hh + 1) * DH, :], zbS[:],
                                                oa[0:DH, :], mybir.AluOpType.mult)
                    for it in range(IQW // P):
                        tg = iq * (IQW // P) + it
                        fin = finp.tile([P, D], F32, tag="fin")
                        for df in range(0, D, IQW):
                            pp = ps.tile([P, IQW], F32, tag="S")
                            nc.tensor.matmul(pp[:], outTq[:, it * P:(it + 1) * P],
                                             wout_t[:, df:df + IQW], start=True, stop=True)
                            nc.vector.tensor_tensor(fin[:, df:df + IQW], bout_bc[:, df:df + IQW],
                                                    pp[:], mybir.AluOpType.add)
                        # out rides the gpsimd queue: on the sync queue its
                        # fin-wait would head-of-line block later bias tiles
                        nc.gpsimd.dma_start(out[b, tg], fin[:])

            prev = None  # (iq, oacc) awaiting tail emission
            for iq in range(IQ):
                isl = slice(iq * IQW, (iq + 1) * IQW)
                oacc = {}
                for b in range(B):
                    for hh in range(NH):
                        oacc[(b, hh)] = oaccp.tile([DH + 1, IQW], F32, tag="oacc",
                                                   name=f"oacc{iq}_{b}{hh}")
                for jt in range(JT):
                    for hh in range(NH):
                        bn = (iq * JT + jt) * NH + hh
                        bias_sb = bias_tiles[bn]
                        issue_bias(bn + PF)
                        lo = hh * DH
                        use_pe = bias_on_pe(bn)
                        for b in range(B):
                            # Balance engines: 3/4 of bias tiles add on the
                            # DVE; 1/4 inject into PSUM via a bf16 identity
                            # matmul (1 cycle/row) and exp straight from PSUM.
                            S = ps.tile([P, IQW], F32, tag="S")
                            if use_pe:
                                nc.tensor.matmul(S[:], ident_b[:], bias_sb[:],
                                                 start=True, stop=False,
                                                 skip_group_check=True)
                            nc.tensor.matmul(S[:], kt[b][lo:lo + DH, jt * P:(jt + 1) * P],
                                             qt[b][lo:lo + DH, isl],
                                             start=not use_pe, stop=True,
                                             skip_group_check=True)
                            pT = ptp.tile([P, IQW], BF16, tag="pT")
                            if use_pe:
                                nc.scalar.activation(pT[:], S[:],
                                                     mybir.ActivationFunctionType.Exp)
                            else:
                                tmp = tmpp.tile([P, IQW], F32, tag="tmp")
                                nc.vector.tensor_tensor(tmp[:], bias_sb[:], S[:],
                                                        mybir.AluOpType.add)
                                nc.scalar.activation(pT[:], tmp[:],
                                                     mybir.ActivationFunctionType.Exp)
                            pending.append((oacc[(b, hh)], vaug[(b, hh)], jt, pT))
                            if len(pending) > DEPTH:
                                flush_one()
                    # Deferred tail: must land before the first oacc flush of
                    # this chunk (the oacc PSUM ring aliases the previous
                    # chunk's accumulators).
                    if jt == 0 and prev is not None:
                        emit_tail(*prev)
                        prev = None
                while pending:
                    flush_one()
                prev = (iq, oacc)
            emit_tail(*prev)

    nc.compile()
    return nc


def make_in_maps(x, mask, rotary_emb, attn_bias, value_residual, Wq, Wkv, Wmix, Wout, bout):
    """Shard + lay out the full inputs for the 8 cores. Layout only, no math."""
    x = np.asarray(x); rotary_emb = np.asarray(rotary_emb)
    attn_bias = np.asarray(attn_bias); value_residual = np.asarray(value_residual)
    Wq = np.asarray(Wq); Wkv = np.asarray(Wkv); Wmix = np.asarray(Wmix)
    Wout = np.asarray(Wout); bout = np.asarray(bout)

    xt_pre = np.ascontiguousarray(
        x.transpose(0, 2, 1).reshape(B, D // P, P, N).transpose(0, 2, 1, 3))
    rott = np.ascontiguousarray(rotary_emb.T)

    def wslice(Wcols):  # [1024, 128 or NH] -> [128, 8, M]
        M = Wcols.shape[1]
        return np.ascontiguousarray(Wcols.reshape(D // P, P, M).transpose(1, 0, 2))

    in_maps = []
    for c in range(NC):
        h0 = NH * c
        hs = slice(h0, h0 + NH)
        # bias transposed to [h, j, i], tiled [h, jt, iq, 128(j), 512(i)]
        biasT = attn_bias[hs].transpose(0, 2, 1)  # [NH, j, i]
        biasTp = np.ascontiguousarray(
            biasT.reshape(NH, JT, P, IQ, IQW).transpose(0, 1, 3, 2, 4))
        vrp = np.ascontiguousarray(
            value_residual[:, hs].reshape(B, NH, NCH, P, DH).transpose(0, 1, 3, 2, 4))
        in_maps.append({
            "xt": xt_pre,
            "wq": wslice(Wq[:, h0 * DH:(h0 + NH) * DH]),
            "wk": wslice(Wkv[:, h0 * DH:(h0 + NH) * DH]),
            "wv": wslice(Wkv[:, H * DH + h0 * DH: H * DH + (h0 + NH) * DH]),
            "wmix": wslice(Wmix[:, hs]),
            "wout": np.ascontiguousarray(Wout[h0 * DH:(h0 + NH) * DH, :]),
            "boutp": (bout if c == 0 else np.zeros_like(bout)).reshape(1, D),
            "rott": rott,
            "biasTp": biasTp,
            "vrp": vrp,
        })
    return in_maps


def unshard(results):
    full = np.zeros((B, NCH, P, D), np.float32)
    for r in results:
        full += r["out"]
    return full.reshape(B, N, D)


_NC_CACHE = None


def kernel(**inputs):
    global _NC_CACHE
    from concourse.bass_utils import run_bass_kernel_spmd
    if _NC_CACHE is None:
        _NC_CACHE = build_nc()
    in_maps = make_in_maps(**inputs)
    res = run_bass_kernel_spmd(_NC_CACHE, in_maps, core_ids=list(range(NC)))
    return unshard(res.results)
